# revision 5
# baseline (speedup 1.0000x reference)
"""Multi-head self-attention (B=1, S=4096, D=512, H=8) on 8 trn2 NeuronCores.

Sharding: one head per core (head/tensor parallel). Each core computes its
head's Q/K/V projections from the full (transposed) query, runs attention
without materializing the full score matrix (streaming over key chunks,
softmax denominator via a ones-column augmented V^T), applies its slice of
out_proj fused with softmax normalization, and writes an unnormalized partial
[S, D] output. Host sums the 8 partials and adds out_proj bias.

All matmuls run in float32r (TF32-like PE fast path, 1 cycle/row at N>=512).
"""

import sys

sys.path.insert(0, "/opt/trn_rl_repo")

import numpy as np

EMBED = 512
HEADS = 8
HD = 64          # head dim
S = 4096         # sequence length
P = 128          # partitions
NSK = S // P     # 32 key chunks of 128
QG = 512         # query group width (matmul free dim)
NQG = S // QG    # 8 query groups
NDC = EMBED // P # 4 contraction chunks for projections
SCALE = HD ** -0.5
EXP_BATCH = 3    # key chunks per exp batch (3 PSUM banks)

_compiled = {}


def _build(n_cores=8, repeats=1):
    import concourse.bacc as bacc
    import concourse.mybir as mybir
    import concourse.tile as tile

    f32 = mybir.dt.float32
    f32r = mybir.dt.float32r
    Exp = mybir.ActivationFunctionType.Exp

    nc = bacc.Bacc("TRN2", target_bir_lowering=False, debug=False,
                   num_devices=n_cores)

    qt = nc.dram_tensor("qt", [EMBED, S], f32r, kind="ExternalInput")
    wq = nc.dram_tensor("wq", [EMBED, HD], f32r, kind="ExternalInput")
    wk = nc.dram_tensor("wk", [EMBED, HD], f32r, kind="ExternalInput")
    wv = nc.dram_tensor("wv", [EMBED, HD], f32r, kind="ExternalInput")
    wo = nc.dram_tensor("wo", [HD, EMBED], f32r, kind="ExternalInput")
    bq = nc.dram_tensor("bq", [HD, 1], f32, kind="ExternalInput")
    bk = nc.dram_tensor("bk", [HD, 1], f32, kind="ExternalInput")
    bv = nc.dram_tensor("bv", [P, HD], f32, kind="ExternalInput")
    out_p = nc.dram_tensor("out_p", [S, EMBED], f32, kind="ExternalOutput")

    with tile.TileContext(nc) as tc:
        for _ in range(repeats):
            _emit(tc, nc, mybir, qt, wq, wk, wv, wo, bq, bk, bv, out_p,
                  f32, f32r, Exp)

    nc.compile()
    return nc


def _emit(tc, nc, mybir, qt, wq, wk, wv, wo, bq, bk, bv, out_p, f32, f32r, Exp):
    from contextlib import ExitStack

    with ExitStack() as ctx:
        singles = ctx.enter_context(tc.tile_pool(name="singles", bufs=1))

        # --- warm up the ACT exp table while DMAs run ---
        warm = singles.tile([1, 1], f32)
        nc.vector.memset(warm, 0.0)
        warm2 = singles.tile([1, 1], f32)
        nc.scalar.activation(warm2, warm, Exp)

        # --- stage A: load weights + transposed query ---
        wq_sb = singles.tile([P, NDC, HD], f32r)
        wk_sb = singles.tile([P, NDC, HD], f32r)
        wv_sb = singles.tile([P, NDC, HD], f32r)
        for c in range(NDC):
            nc.sync.dma_start(out=wq_sb[:, c, :], in_=wq[c * P:(c + 1) * P, :])
            nc.sync.dma_start(out=wk_sb[:, c, :], in_=wk[c * P:(c + 1) * P, :])
            nc.sync.dma_start(out=wv_sb[:, c, :], in_=wv[c * P:(c + 1) * P, :])
        wo_sb = singles.tile([HD, EMBED], f32r)
        nc.sync.dma_start(out=wo_sb, in_=wo[:, :])
        bq_sb = singles.tile([HD, 1], f32)
        nc.sync.dma_start(out=bq_sb, in_=bq[:, :])
        bk_sb = singles.tile([HD, 1], f32)
        nc.sync.dma_start(out=bk_sb, in_=bk[:, :])
        bv_sb = singles.tile([P, HD], f32)
        nc.sync.dma_start(out=bv_sb, in_=bv[:, :])

        qt_sb = []
        for c in range(NDC):
            t = singles.tile([P, S], f32r, tag=f"qt{c}")
            nc.sync.dma_start(out=t, in_=qt[c * P:(c + 1) * P, :])
            qt_sb.append(t)

        # persistent activations
        q_sb = singles.tile([HD, S], f32r)     # Q^T per head: [hd, s]
        k_sb = singles.tile([HD, S], f32r)     # K^T per head: [hd, s]
        vt_sb = singles.tile([P, NSK, HD + 1], f32r)  # V^T chunks + ones col
        ot_sb = singles.tile([HD, S], f32r)    # unnormalized attn out^T
        recip_row = singles.tile([1, S], f32)  # 1/denominator, row layout
        recip_all = singles.tile([P, NSK], f32)  # 1/denominator, [sq%128, chunk]

        for s in range(NSK):
            nc.vector.memset(vt_sb[:, s, HD:HD + 1].bitcast(f32), 1.0)

        # --- stage B: projections ---
        with ExitStack() as bctx:
            pqk = bctx.enter_context(
                tc.tile_pool(name="pqk", bufs=2, space="PSUM"))
            pvp = bctx.enter_context(
                tc.tile_pool(name="pvp", bufs=2, space="PSUM"))

            for g in range(NQG):
                sl = slice(g * QG, (g + 1) * QG)
                acc_q = pqk.tile([HD, QG], f32, tag="pj")
                for c in range(NDC):
                    nc.tensor.matmul(acc_q, wq_sb[:, c, :], qt_sb[c][:, sl],
                                     start=(c == 0), stop=(c == NDC - 1))
                nc.vector.tensor_scalar_add(q_sb[:, sl], acc_q, bq_sb)
                acc_k = pqk.tile([HD, QG], f32, tag="pj")
                for c in range(NDC):
                    nc.tensor.matmul(acc_k, wk_sb[:, c, :], qt_sb[c][:, sl],
                                     start=(c == 0), stop=(c == NDC - 1))
                nc.vector.tensor_scalar_add(k_sb[:, sl], acc_k, bk_sb)

            for s in range(NSK):
                ssl = slice(s * P, (s + 1) * P)
                acc_v = pvp.tile([P, HD], f32, tag="pv")
                for c in range(NDC):
                    nc.tensor.matmul(acc_v, qt_sb[c][:, ssl], wv_sb[:, c, :],
                                     start=(c == 0), stop=(c == NDC - 1))
                nc.vector.tensor_add(vt_sb[:, s, 0:HD], acc_v, bv_sb)

        # --- stage C: attention, streaming over key chunks ---
        with ExitStack() as cctx:
            s_pool = cctx.enter_context(
                tc.tile_pool(name="s_pool", bufs=2, space="PSUM"))
            acc_pool = cctx.enter_context(
                tc.tile_pool(name="acc_pool", bufs=2, space="PSUM"))
            p_pool = cctx.enter_context(tc.tile_pool(name="p_pool", bufs=3))
            o_pool = cctx.enter_context(tc.tile_pool(name="o_pool", bufs=3))

            n_batches = (NSK + EXP_BATCH - 1) // EXP_BATCH
            for g in range(NQG):
                gsl = slice(g * QG, (g + 1) * QG)
                out_acc = acc_pool.tile([HD + 1, QG], f32, tag="acc")
                for b in range(n_batches):
                    chunks = list(range(b * EXP_BATCH,
                                        min((b + 1) * EXP_BATCH, NSK)))
                    nb = len(chunks)
                    s_ps = s_pool.tile([P, EXP_BATCH * QG], f32, tag="sps")
                    for i, s in enumerate(chunks):
                        nc.tensor.matmul(
                            s_ps[:, i * QG:(i + 1) * QG],
                            k_sb[:, s * P:(s + 1) * P], q_sb[:, gsl],
                            start=True, stop=True)
                    p_sb = p_pool.tile([P, EXP_BATCH * QG], f32r, tag="p")
                    nc.scalar.activation(p_sb[:, :nb * QG], s_ps[:, :nb * QG],
                                         Exp, scale=SCALE)
                    for i, s in enumerate(chunks):
                        nc.tensor.matmul(
                            out_acc, vt_sb[:, s, :],
                            p_sb[:, i * QG:(i + 1) * QG],
                            start=(s == 0), stop=(s == NSK - 1))
                # evict: numerator rows (f32r for out-proj), denominator row
                nc.vector.tensor_copy(ot_sb[:, gsl], out_acc[0:HD, :])
                nc.vector.reciprocal(recip_row[:, gsl],
                                     out_acc[HD:HD + 1, :])
                # transpose [1, 512] -> [128, 4] so normalization is
                # per-partition in the output layout
                for i in range(QG // P):
                    j = g * (QG // P) + i
                    nc.sync.dma_start(
                        out=recip_all[:, j:j + 1],
                        in_=recip_row[0:1, j * P:(j + 1) * P])

            # --- stage D: out_proj slice + normalization ---
            for t in range(NSK):
                tsl = slice(t * P, (t + 1) * P)
                o_ps = acc_pool.tile([P, EMBED], f32, tag="acc")
                nc.tensor.matmul(o_ps, ot_sb[:, tsl], wo_sb,
                                 start=True, stop=True)
                o_sb = o_pool.tile([P, EMBED], f32, tag="o")
                nc.vector.tensor_scalar_mul(o_sb, o_ps, recip_all[:, t:t + 1])
                nc.sync.dma_start(out=out_p[tsl, :], in_=o_sb)


def _in_maps(query, in_proj_weight, in_proj_bias, out_proj_weight):
    q2d = np.asarray(query, dtype=np.float32).reshape(S, EMBED)
    qt = np.ascontiguousarray(q2d.T)
    w = np.asarray(in_proj_weight, dtype=np.float32)
    b = np.asarray(in_proj_bias, dtype=np.float32)
    wout = np.asarray(out_proj_weight, dtype=np.float32)
    maps = []
    for h in range(HEADS):
        hs = slice(h * HD, (h + 1) * HD)
        maps.append({
            "qt": qt,
            "wq": np.ascontiguousarray(w[hs, :].T),
            "wk": np.ascontiguousarray(w[EMBED + h * HD:EMBED + (h + 1) * HD, :].T),
            "wv": np.ascontiguousarray(w[2 * EMBED + h * HD:2 * EMBED + (h + 1) * HD, :].T),
            "wo": np.ascontiguousarray(wout[:, hs].T),
            "bq": np.ascontiguousarray(b[hs].reshape(HD, 1)),
            "bk": np.ascontiguousarray(b[EMBED + h * HD:EMBED + (h + 1) * HD].reshape(HD, 1)),
            "bv": np.ascontiguousarray(
                np.broadcast_to(b[2 * EMBED + h * HD:2 * EMBED + (h + 1) * HD], (P, HD))),
        })
    return maps


def get_nc():
    if "nc" not in _compiled:
        _compiled["nc"] = _build()
    return _compiled["nc"]


def kernel(query, in_proj_weight, in_proj_bias, out_proj_weight, out_proj_bias):
    from concourse.bass_utils import run_bass_kernel_spmd

    nc = get_nc()
    maps = _in_maps(query, in_proj_weight, in_proj_bias, out_proj_weight)
    res = run_bass_kernel_spmd(nc, maps, core_ids=list(range(HEADS)))
    acc = np.zeros((S, EMBED), dtype=np.float32)
    for h in range(HEADS):
        acc += res.results[h]["out_p"]
    acc += np.asarray(out_proj_bias, dtype=np.float32)[None, :]
    return acc.reshape(np.asarray(query).shape).astype(np.float32)


# revision 6
# speedup vs baseline: 2.1490x; 2.1490x over previous
"""Multi-head self-attention (B=1, S=4096, D=512, H=8) on 8 trn2 NeuronCores.

Sharding: one head per core (head/tensor parallel). Each core computes its
head's Q/K/V projections from the full (transposed) query, runs attention
without materializing the full score matrix (streaming over key chunks,
softmax denominator via a ones-column augmented V^T), applies its slice of
out_proj fused with softmax normalization, and writes an unnormalized partial
[S, D] output. Host sums the 8 partials and adds out_proj bias.

All matmuls run in float32r (TF32-like PE fast path, 1 cycle/row at N>=512).
"""

import sys

sys.path.insert(0, "/opt/trn_rl_repo")

import numpy as np

EMBED = 512
HEADS = 8
HD = 64          # head dim
S = 4096         # sequence length
P = 128          # partitions
NSK = S // P     # 32 key chunks of 128
QG = 512         # query group width (matmul free dim)
NQG = S // QG    # 8 query groups
NDC = EMBED // P # 4 contraction chunks for projections
SCALE = HD ** -0.5
EXP_BATCH = 3    # key chunks per exp batch (3 PSUM banks)

_compiled = {}


def _build(n_cores=8, repeats=1, stages="ABCD"):
    import concourse.bacc as bacc
    import concourse.mybir as mybir
    import concourse.tile as tile

    f32 = mybir.dt.float32
    f32r = mybir.dt.float32r
    Exp = mybir.ActivationFunctionType.Exp

    nc = bacc.Bacc("TRN2", target_bir_lowering=False, debug=False,
                   num_devices=n_cores)

    qt = nc.dram_tensor("qt", [EMBED, S], f32r, kind="ExternalInput")
    wq = nc.dram_tensor("wq", [EMBED, HD], f32r, kind="ExternalInput")
    wk = nc.dram_tensor("wk", [EMBED, HD], f32r, kind="ExternalInput")
    wv = nc.dram_tensor("wv", [EMBED, HD], f32r, kind="ExternalInput")
    wo = nc.dram_tensor("wo", [HD, EMBED], f32r, kind="ExternalInput")
    bq = nc.dram_tensor("bq", [HD, 1], f32, kind="ExternalInput")
    bk = nc.dram_tensor("bk", [HD, 1], f32, kind="ExternalInput")
    bv = nc.dram_tensor("bv", [P, HD], f32, kind="ExternalInput")
    out_p = nc.dram_tensor("out_p", [S, EMBED], f32, kind="ExternalOutput")

    with tile.TileContext(nc) as tc:
        for _ in range(repeats):
            _emit(tc, nc, mybir, qt, wq, wk, wv, wo, bq, bk, bv, out_p,
                  f32, f32r, Exp, stages)

    nc.compile()
    return nc


def _emit(tc, nc, mybir, qt, wq, wk, wv, wo, bq, bk, bv, out_p, f32, f32r, Exp,
          stages="ABCD"):
    from contextlib import ExitStack

    with ExitStack() as ctx:
        singles = ctx.enter_context(tc.tile_pool(name="singles", bufs=1))

        # --- warm up the ACT exp table while DMAs run ---
        warm = singles.tile([1, 1], f32)
        nc.vector.memset(warm, 0.0)
        warm2 = singles.tile([1, 1], f32)
        nc.scalar.activation(warm2, warm, Exp)

        # --- stage A: load weights + transposed query ---
        wq_sb = singles.tile([P, NDC, HD], f32r)
        wk_sb = singles.tile([P, NDC, HD], f32r)
        wv_sb = singles.tile([P, NDC, HD], f32r)
        for c in range(NDC):
            nc.sync.dma_start(out=wq_sb[:, c, :], in_=wq[c * P:(c + 1) * P, :])
            nc.sync.dma_start(out=wk_sb[:, c, :], in_=wk[c * P:(c + 1) * P, :])
            nc.sync.dma_start(out=wv_sb[:, c, :], in_=wv[c * P:(c + 1) * P, :])
        wo_sb = singles.tile([HD, EMBED], f32r)
        nc.sync.dma_start(out=wo_sb, in_=wo[:, :])
        bq_sb = singles.tile([HD, 1], f32)
        nc.sync.dma_start(out=bq_sb, in_=bq[:, :])
        bk_sb = singles.tile([HD, 1], f32)
        nc.sync.dma_start(out=bk_sb, in_=bk[:, :])
        bv_sb = singles.tile([P, HD], f32)
        nc.sync.dma_start(out=bv_sb, in_=bv[:, :])

        qt_sb = []
        for c in range(NDC):
            t = singles.tile([P, S], f32r, tag=f"qt{c}")
            nc.sync.dma_start(out=t, in_=qt[c * P:(c + 1) * P, :])
            qt_sb.append(t)

        # persistent activations
        q_sb = singles.tile([HD, S], f32r)     # Q^T per head: [hd, s]
        k_sb = singles.tile([HD, S], f32r)     # K^T per head: [hd, s]
        vt_sb = singles.tile([P, NSK, HD + 1], f32r)  # V^T chunks + ones col
        ot_sb = singles.tile([HD, S], f32r)    # unnormalized attn out^T
        recip_row = singles.tile([1, S], f32)  # 1/denominator, row layout
        recip_all = singles.tile([P, NSK], f32)  # 1/denominator, [sq%128, chunk]

        for s in range(NSK):
            nc.vector.memset(vt_sb[:, s, HD:HD + 1].bitcast(f32), 1.0)

        if "B" not in stages:
            for t_ in (q_sb, k_sb, ot_sb):
                nc.vector.memset(t_[:, 0:S].bitcast(f32), 0.001)
            for s in range(NSK):
                nc.vector.memset(vt_sb[:, s, 0:HD].bitcast(f32), 0.001)
            nc.vector.memset(recip_row[:, 0:S], 1.0)
            nc.vector.memset(recip_all, 1.0)

        # --- stage B: projections ---
        with ExitStack() as bctx:
            if "B" not in stages:
                bctx = None
        if "B" in stages:
          with ExitStack() as bctx:
            pqk = bctx.enter_context(
                tc.tile_pool(name="pqk", bufs=2, space="PSUM"))
            pvp = bctx.enter_context(
                tc.tile_pool(name="pvp", bufs=2, space="PSUM"))

            for g in range(NQG):
                sl = slice(g * QG, (g + 1) * QG)
                acc_q = pqk.tile([HD, QG], f32, tag="pj")
                for c in range(NDC):
                    nc.tensor.matmul(acc_q, wq_sb[:, c, :], qt_sb[c][:, sl],
                                     start=(c == 0), stop=(c == NDC - 1))
                nc.vector.tensor_scalar_add(q_sb[:, sl], acc_q, bq_sb)
                acc_k = pqk.tile([HD, QG], f32, tag="pj")
                for c in range(NDC):
                    nc.tensor.matmul(acc_k, wk_sb[:, c, :], qt_sb[c][:, sl],
                                     start=(c == 0), stop=(c == NDC - 1))
                nc.vector.tensor_scalar_add(k_sb[:, sl], acc_k, bk_sb)

            for s in range(NSK):
                ssl = slice(s * P, (s + 1) * P)
                acc_v = pvp.tile([P, HD], f32, tag="pv")
                for c in range(NDC):
                    nc.tensor.matmul(acc_v, qt_sb[c][:, ssl], wv_sb[:, c, :],
                                     start=(c == 0), stop=(c == NDC - 1))
                nc.vector.tensor_add(vt_sb[:, s, 0:HD], acc_v, bv_sb)

        if "Z" in stages:
            return
        # --- stage C: attention, streaming over key chunks ---
        with ExitStack() as cctx:
            s_pool = cctx.enter_context(
                tc.tile_pool(name="s_pool", bufs=2, space="PSUM"))
            acc_pool = cctx.enter_context(
                tc.tile_pool(name="acc_pool", bufs=2, space="PSUM"))
            p_pool = cctx.enter_context(tc.tile_pool(name="p_pool", bufs=3))
            o_pool = cctx.enter_context(tc.tile_pool(name="o_pool", bufs=3))

            n_batches = (NSK + EXP_BATCH - 1) // EXP_BATCH
            for g in range(NQG):
                gsl = slice(g * QG, (g + 1) * QG)
                out_acc = acc_pool.tile([HD + 1, QG], f32, tag="acc")
                for b in range(n_batches):
                    chunks = list(range(b * EXP_BATCH,
                                        min((b + 1) * EXP_BATCH, NSK)))
                    nb = len(chunks)
                    s_ps = s_pool.tile([P, EXP_BATCH * QG], f32, tag="sps")
                    for i, s in enumerate(chunks):
                        nc.tensor.matmul(
                            s_ps[:, i * QG:(i + 1) * QG],
                            k_sb[:, s * P:(s + 1) * P], q_sb[:, gsl],
                            start=True, stop=True)
                    p_sb = p_pool.tile([P, EXP_BATCH * QG], f32r, tag="p")
                    nc.scalar.activation(p_sb[:, :nb * QG], s_ps[:, :nb * QG],
                                         Exp, scale=SCALE)
                    for i, s in enumerate(chunks):
                        nc.tensor.matmul(
                            out_acc, vt_sb[:, s, :],
                            p_sb[:, i * QG:(i + 1) * QG],
                            start=(s == 0), stop=(s == NSK - 1))
                # evict: numerator rows (f32r for out-proj), denominator row
                nc.vector.tensor_copy(ot_sb[:, gsl], out_acc[0:HD, :])
                nc.vector.reciprocal(recip_row[:, gsl],
                                     out_acc[HD:HD + 1, :])
                # transpose [1, 512] -> [128, 4] so normalization is
                # per-partition in the output layout
                for i in range(QG // P):
                    j = g * (QG // P) + i
                    nc.sync.dma_start(
                        out=recip_all[:, j:j + 1],
                        in_=recip_row[0:1, j * P:(j + 1) * P])

            # --- stage D: out_proj slice + normalization ---
            for t in range(NSK):
                tsl = slice(t * P, (t + 1) * P)
                o_ps = acc_pool.tile([P, EMBED], f32, tag="acc")
                nc.tensor.matmul(o_ps, ot_sb[:, tsl], wo_sb,
                                 start=True, stop=True)
                o_sb = o_pool.tile([P, EMBED], f32, tag="o")
                nc.vector.tensor_scalar_mul(o_sb, o_ps, recip_all[:, t:t + 1])
                nc.sync.dma_start(out=out_p[tsl, :], in_=o_sb)


def _in_maps(query, in_proj_weight, in_proj_bias, out_proj_weight):
    q2d = np.asarray(query, dtype=np.float32).reshape(S, EMBED)
    qt = np.ascontiguousarray(q2d.T)
    w = np.asarray(in_proj_weight, dtype=np.float32)
    b = np.asarray(in_proj_bias, dtype=np.float32)
    wout = np.asarray(out_proj_weight, dtype=np.float32)
    maps = []
    for h in range(HEADS):
        hs = slice(h * HD, (h + 1) * HD)
        maps.append({
            "qt": qt,
            "wq": np.ascontiguousarray(w[hs, :].T),
            "wk": np.ascontiguousarray(w[EMBED + h * HD:EMBED + (h + 1) * HD, :].T),
            "wv": np.ascontiguousarray(w[2 * EMBED + h * HD:2 * EMBED + (h + 1) * HD, :].T),
            "wo": np.ascontiguousarray(wout[:, hs].T),
            "bq": np.ascontiguousarray(b[hs].reshape(HD, 1)),
            "bk": np.ascontiguousarray(b[EMBED + h * HD:EMBED + (h + 1) * HD].reshape(HD, 1)),
            "bv": np.ascontiguousarray(
                np.broadcast_to(b[2 * EMBED + h * HD:2 * EMBED + (h + 1) * HD], (P, HD))),
        })
    return maps


def get_nc():
    if "nc" not in _compiled:
        _compiled["nc"] = _build()
    return _compiled["nc"]


def kernel(query, in_proj_weight, in_proj_bias, out_proj_weight, out_proj_bias):
    from concourse.bass_utils import run_bass_kernel_spmd

    nc = get_nc()
    maps = _in_maps(query, in_proj_weight, in_proj_bias, out_proj_weight)
    res = run_bass_kernel_spmd(nc, maps, core_ids=list(range(HEADS)))
    acc = np.zeros((S, EMBED), dtype=np.float32)
    for h in range(HEADS):
        acc += res.results[h]["out_p"]
    acc += np.asarray(out_proj_bias, dtype=np.float32)[None, :]
    return acc.reshape(np.asarray(query).shape).astype(np.float32)
